# revision 26
# baseline (speedup 1.0000x reference)
"""Multi-head attention (B=2, T=2048, D=1024, 16 heads) on 8 TRN2 NeuronCores.

Sharding: tensor-parallel over heads (2 heads/core). Each core computes
Q/K/V projections for its 2 heads (full sequence), causal attention in the
S^T = K @ Q^T form, and a partial output projection; the host sums the 8
partial outputs.

v2 redesign vs baseline (293us):
- all matmul operands bf16 (same 1 cycle/row as f32r but FWL weight loads
  and half the HBM traffic; rel-err budget 2e-2 >> bf16 error)
- causal masking via gpsimd affine_select post-exp zeroing (idle engine)
  instead of additive mask tensors on DVE; diagonal blocks are also
  extent-trimmed (S/exp/AV only computed on the valid column range)
- V transposed into [tk, d] layout by DMA XBAR transpose, not PE transpose
- softmax denominators: reciprocal_approx_fast on [1,512] rows + gpsimd
  partition_broadcast (no DRAM bounce)
- two clean phases that each fit the 8 PSUM banks exactly: (1) all QKV
  projections, (2) attention with o-proj steps interleaved via deferral
"""

import sys

sys.path.insert(0, "/opt/trn_rl_repo")

import numpy as np

B, T, D = 2, 2048, 1024
NCORES = 8
DV = 128  # head dims per core (2 heads x 64)
DH = 64
BT = B * T
CH = 512  # tq chunk width
NCH = BT // CH  # 8 global chunks
NCH_B = T // CH  # 4 chunks per batch
TK = 128  # tk tile
NTK = T // TK  # 16 tiles per batch
ND = D // 128  # 8 contraction tiles
DVA = DH + 1  # V columns incl ones column (for softmax denominator)

_cache = {}


def _build(debug=False):
    import concourse.bacc as bacc
    import concourse.mybir as mybir
    import concourse.tile as tile

    F32 = mybir.dt.float32
    BF16 = mybir.dt.bfloat16
    EXP = mybir.ActivationFunctionType.Exp
    MULT = mybir.AluOpType.mult
    from concourse.masks import make_identity

    nc = bacc.Bacc("TRN2", target_bir_lowering=False, debug=False,
                   num_devices=NCORES)

    xt_d = nc.dram_tensor("xt", [D, BT], BF16, kind="ExternalInput").ap()
    wq_d = nc.dram_tensor("wq", [128, D], BF16, kind="ExternalInput").ap()
    wk_d = nc.dram_tensor("wk", [128, D], BF16, kind="ExternalInput").ap()
    wv_d = nc.dram_tensor("wv", [128, D], BF16, kind="ExternalInput").ap()
    wo_d = nc.dram_tensor("wo", [128, D], BF16, kind="ExternalInput").ap()
    o_d = nc.dram_tensor("o", [BT, D], BF16, kind="ExternalOutput").ap()

    NU = B * NTK * 2  # 64 per-head V blocks
    if debug:
        qT_dbg = nc.dram_tensor("qT_dbg", [128, BT], BF16,
                                kind="ExternalOutput").ap()
        kT_dbg = nc.dram_tensor("kT_dbg", [128, BT], BF16,
                                kind="ExternalOutput").ap()
        vsb_dbg = nc.dram_tensor("vsb_dbg", [128, NU * DVA], BF16,
                                 kind="ExternalOutput").ap()
        outT_dbg = nc.dram_tensor("outT_dbg", [128, BT], BF16,
                                  kind="ExternalOutput").ap()
        srow_dbg = nc.dram_tensor("srow_dbg", [2, NCH, CH], F32,
                                  kind="ExternalOutput").ap()
        rbc_dbg = nc.dram_tensor("rbc_dbg", [128, NCH, CH], F32,
                                 kind="ExternalOutput").ap()
        p_dbg = nc.dram_tensor("p_dbg", [128, 4, 2, CH], BF16,
                               kind="ExternalOutput").ap()

    with tile.TileContext(nc) as tc:
        with tc.tile_pool(name="consts", bufs=1) as consts, \
             tc.tile_pool(name="perm", bufs=1) as perm, \
             tc.tile_pool(name="xt_pool", bufs=4) as xtp, \
             tc.tile_pool(name="vtf_pool", bufs=2) as vtfp, \
             tc.tile_pool(name="p_pool", bufs=4) as ppool, \
             tc.tile_pool(name="avc_pool", bufs=2) as avcp, \
             tc.tile_pool(name="rec_pool", bufs=2) as recp, \
             tc.tile_pool(name="outT_pool", bufs=2) as outTp, \
             tc.tile_pool(name="osb_pool", bufs=3) as obp, \
             tc.tile_pool(name="dram_pool", bufs=2, space="DRAM") as drp:
            wq_sb = consts.tile([128, D], BF16, name="wq_sb")
            wk_sb = consts.tile([128, D], BF16, name="wk_sb")
            wv_sb = consts.tile([128, D], BF16, name="wv_sb")
            wo_sb = consts.tile([128, D], BF16, name="wo_sb")
            ident = consts.tile([128, 128], BF16, name="ident")
            make_identity(nc, ident[:])

            qT = perm.tile([128, BT], BF16, name="qT")
            kT = perm.tile([128, BT], BF16, name="kT")
            # V blocks, t-major with ones column: per (b, tile, head) a
            # [128(tk), 65] block at free offset u*65, u = (b*NTK+i)*2+h
            vsb = perm.tile([128, NU * DVA], BF16, name="vsb")
            nc.gpsimd.memset(
                vsb[:].rearrange("p (u c) -> p u c", c=DVA)[:, :, DH:DVA], 1.0)


            # ---------------- Phase 1: all QKV projections ----------------
            for jp in range(NCH // 2):
                j0 = 2 * jp
                with tc.tile_pool(name=f"proj_ps{jp}", bufs=1,
                                  space="PSUM") as pps, \
                     tc.tile_pool(name=f"vt_ps{jp}", bufs=2,
                                  space="PSUM") as vtps:
                    acc = {}
                    for nm in ("q", "k", "v"):
                        for half in (0, 1):
                            acc[(nm, half)] = pps.tile(
                                [128, CH], F32, tag=f"{nm}{half}",
                                name=f"{nm}ps{j0 + half}")
                    for d in range(ND):
                        xt = xtp.tile([128, 2 * CH], BF16, tag="xt",
                                      name=f"xt{jp}_{d}")
                        nc.sync.dma_start(
                            xt[:], xt_d[d * 128:(d + 1) * 128,
                                        j0 * CH:(j0 + 2) * CH])
                        st, sp = d == 0, d == ND - 1
                        ws = slice(d * 128, (d + 1) * 128)
                        if jp == 0:
                            nc.sync.dma_start(wq_sb[:, ws], wq_d[:, ws])
                            nc.sync.dma_start(wk_sb[:, ws], wk_d[:, ws])
                            nc.sync.dma_start(wv_sb[:, ws], wv_d[:, ws])
                        for nm, w_sb in (("q", wq_sb), ("k", wk_sb),
                                         ("v", wv_sb)):
                            for half in (0, 1):
                                nc.tensor.matmul(
                                    acc[(nm, half)][:], w_sb[:, ws],
                                    xt[:, half * CH:(half + 1) * CH],
                                    start=st, stop=sp)
                    for half in (0, 1):
                        j = j0 + half
                        cs = slice(j * CH, (j + 1) * CH)
                        vtf = vtfp.tile([128, CH], BF16, tag="vtf",
                                        name=f"vtf{j}")
                        if half == 0:
                            nc.vector.tensor_copy(vtf[:], acc[("v", half)][:])
                        else:
                            nc.scalar.copy(vtf[:], acc[("v", half)][:])
                        nc.vector.tensor_copy(qT[:, cs], acc[("q", half)][:])
                        nc.scalar.copy(kT[:, cs], acc[("k", half)][:])
                        for tt in range(4):
                            tglob = 4 * j + tt
                            bb, ii = tglob // NTK, tglob % NTK
                            u0 = (bb * NTK + ii) * 2
                            vt_ps = vtps.tile([128, 128], BF16, tag="vt",
                                              name=f"vt{tglob}")
                            nc.tensor.transpose(
                                vt_ps[:], vtf[:, tt * 128:(tt + 1) * 128],
                                ident[:])
                            dst = vsb[:, u0 * DVA:(u0 + 2) * DVA].rearrange(
                                "p (h c) -> p h c", c=DVA)[:, :, 0:DH]
                            nc.vector.tensor_copy(
                                dst, vt_ps[:].rearrange("p (h c) -> p h c",
                                                        c=DH))
                if jp == 0:
                    nc.sync.dma_start(wo_sb[:], wo_d[:])

            # ---------------- Phase 2: attention + o-proj ----------------
            deferred = []

            def emit_oproj(b, jj, outT):
                for tt in range(4):
                    osb = obp.tile([128, D], BF16, tag="osb",
                                   name=f"osb{b}_{jj}_{tt}")
                    for half in (0, 1):
                        def step(opsp, tt=tt, half=half, b=b, jj=jj,
                                 outT=outT, osb=osb):
                            op = opsp.tile([128, CH], F32, tag="op",
                                           name=f"op{b}_{jj}_{tt}_{half}")
                            ts = slice(tt * 128, (tt + 1) * 128)
                            hs = slice(half * CH, (half + 1) * CH)
                            nc.tensor.matmul(op[:], outT[:, ts],
                                             wo_sb[:, hs],
                                             start=True, stop=True)
                            nc.any.tensor_copy(osb[:, hs], op[:])
                            if half == 1:
                                r0 = b * T + jj * CH + tt * 128
                                nc.sync.dma_start(o_d[r0:r0 + 128, :],
                                                  osb[:])
                        deferred.append(step)

            def attention_chunk(b, jj, spsp, avp, opsp):
                kept = list(range(4 * (jj + 1)))
                av0 = avp.tile([DVA, CH], F32, tag="av0", name=f"av0_{b}_{jj}",
                               bufs=2)
                av1 = avp.tile([DVA, CH], F32, tag="av1", name=f"av1_{b}_{jj}",
                               bufs=1)
                tq0 = (b * NCH_B + jj) * CH
                pend = None

                def emit_av(i, p, n0):
                    st = i == kept[0]
                    sp = i == kept[-1]
                    u0 = (b * NTK + i) * 2
                    nc.tensor.matmul(
                        av0[:, n0:CH], vsb[:, u0 * DVA:u0 * DVA + DVA],
                        p[:, 0, n0:CH], start=st, stop=sp)
                    nc.tensor.matmul(
                        av1[:, n0:CH], vsb[:, (u0 + 1) * DVA:(u0 + 2) * DVA],
                        p[:, 1, n0:CH], start=st, stop=sp)

                for i in kept:
                    r = i - 4 * jj  # diagonal sub-block index (>=0: diagonal)
                    n0 = 128 * r if r > 0 else 0  # first valid tq column
                    ks = slice((b * NTK + i) * TK, (b * NTK + i + 1) * TK)
                    sps = spsp.tile([128, 2, CH], F32, tag="sps",
                                    name=f"sps{b}_{jj}_{i}")
                    nc.tensor.matmul(sps[:, 0, n0:CH], kT[0:64, ks],
                                     qT[0:64, tq0 + n0:tq0 + CH],
                                     start=True, stop=True)
                    nc.tensor.matmul(sps[:, 1, n0:CH], kT[64:128, ks],
                                     qT[64:128, tq0 + n0:tq0 + CH],
                                     start=True, stop=True)
                    p = ppool.tile([128, 2, CH], BF16, tag="p",
                                   name=f"p{b}_{jj}_{i}")
                    nc.scalar.activation(p[:, :, n0:CH], sps[:, :, n0:CH],
                                         EXP)
                    if r >= 0:
                        # zero the strict upper triangle of the diagonal
                        # [128,128] sub-block: keep iff tk(partition) <= tq
                        nc.gpsimd.affine_select(
                            out=p[:, :, n0:n0 + 128],
                            in_=p[:, :, n0:n0 + 128],
                            compare_op=mybir.AluOpType.is_ge,
                            fill=0.0,
                            base=0,
                            pattern=[[0, 2], [1, 128]],
                            channel_multiplier=-1,
                        )
                    if debug and b == 0 and jj == 0:
                        nc.sync.dma_start(p_dbg[:, i, :, :], p[:])
                    if deferred:
                        deferred.pop(0)(opsp)
                        if len(deferred) > 6:
                            deferred.pop(0)(opsp)
                    if pend is not None:
                        emit_av(*pend)
                    pend = (i, p, n0)
                emit_av(*pend)

                # evacuate av banks (row DH holds the softmax denominators);
                # partition-crossing copies are legal, compute ops must stay
                # partition-aligned
                avc = avcp.tile([128, CH], F32, tag="avc",
                                name=f"avc_{b}_{jj}")
                srowA = avcp.tile([1, CH], F32, tag="srowA",
                                  name=f"srowA_{b}_{jj}")
                srowB = avcp.tile([1, CH], F32, tag="srowB",
                                  name=f"srowB_{b}_{jj}")
                # av1 is single-buffered: evacuate it first, on the scalar
                # engine, so the next chunk's AV can start promptly
                nc.any.tensor_copy(avc[64:128, :], av1[0:DH, :])
                nc.any.tensor_copy(srowB[:], av1[DH:DVA, :])
                nc.any.tensor_copy(avc[0:64, :], av0[0:DH, :])
                nc.any.tensor_copy(srowA[:], av0[DH:DVA, :])
                recA = recp.tile([1, CH], F32, tag="recA", name=f"recA{b}_{jj}")
                recB = recp.tile([1, CH], F32, tag="recB", name=f"recB{b}_{jj}")
                nc.vector.reciprocal_approx_fast(recA[:], srowA[:])
                nc.vector.reciprocal_approx_fast(recB[:], srowB[:])
                dr = drp.tile([2, CH], F32, tag="dr", name=f"dr_{b}_{jj}")
                nc.sync.dma_start(dr[0:1, :], recA[:])
                nc.sync.dma_start(dr[1:2, :], recB[:])
                rbc = recp.tile([128, CH], F32, tag="rbc", name=f"rbc{b}_{jj}")
                nc.sync.dma_start(rbc[0:64, :],
                                  dr[0:1, :].broadcast_to([64, CH]))
                nc.sync.dma_start(rbc[64:128, :],
                                  dr[1:2, :].broadcast_to([64, CH]))
                outT = outTp.tile([128, CH], BF16, tag="outT",
                                  name=f"outT{b}_{jj}")
                nc.gpsimd.tensor_tensor(out=outT[0:64, :], in0=avc[0:64, :],
                                         in1=rbc[0:64, :], op=MULT)
                nc.gpsimd.tensor_tensor(out=outT[64:128, :],
                                        in0=avc[64:128, :],
                                        in1=rbc[64:128, :], op=MULT)
                if debug:
                    cidx = b * NCH_B + jj
                    cs_ = slice(cidx * CH, (cidx + 1) * CH)
                    nc.sync.dma_start(outT_dbg[:, cs_], outT[:])
                    nc.sync.dma_start(srow_dbg[0:1, cidx], srowA[:])
                    nc.sync.dma_start(srow_dbg[1:2, cidx], srowB[:])
                    nc.sync.dma_start(rbc_dbg[:, cidx], rbc[:])
                emit_oproj(b, jj, outT)

            with tc.tile_pool(name="s_ps", bufs=2, space="PSUM") as spsp, \
                 tc.tile_pool(name="av_ps", bufs=2, space="PSUM") as avp, \
                 tc.tile_pool(name="o_ps", bufs=1, space="PSUM") as opsp:
                for b in range(B):
                    for jj in range(NCH_B):
                        attention_chunk(b, jj, spsp, avp, opsp)
                while deferred:
                    deferred.pop(0)(opsp)
                if debug:
                    nc.sync.dma_start(qT_dbg[:], qT[:])
                    nc.sync.dma_start(kT_dbg[:], kT[:])
                    nc.sync.dma_start(vsb_dbg[:], vsb[:])

    nc.compile()
    return nc


def kernel(x, Wq, Wk, Wv, Wo, attn_mask):
    import concourse.bass_utils as _bu
    import ml_dtypes
    run_bass_kernel_spmd = _bu.run_bass_kernel_spmd
    BF = ml_dtypes.bfloat16

    x = np.asarray(x, dtype=np.float32)
    Wq = np.asarray(Wq, dtype=np.float32)
    Wk = np.asarray(Wk, dtype=np.float32)
    Wv = np.asarray(Wv, dtype=np.float32)
    Wo = np.asarray(Wo, dtype=np.float32)

    xT = np.ascontiguousarray(x.reshape(BT, D).T).astype(BF)

    import os
    dbg = bool(os.environ.get("MHA_DEBUG"))
    if ("nc", dbg) not in _cache:
        _cache[("nc", dbg)] = _build(debug=dbg)
    nc = _cache[("nc", dbg)]

    in_maps = []
    for c in range(NCORES):
        rows = slice(c * DV, (c + 1) * DV)

        def wlayout(W, scale=1.0):
            Wc = W[rows, :]  # [128, D]
            return np.ascontiguousarray(
                (Wc.T.reshape(ND, 128, 128).transpose(1, 0, 2)
                 .reshape(128, D) * scale)).astype(BF)

        wo_dev = np.ascontiguousarray(Wo[:, rows].T).astype(BF)
        in_maps.append({
            "xt": xT,
            "wq": wlayout(Wq, 0.125),
            "wk": wlayout(Wk),
            "wv": wlayout(Wv),
            "wo": wo_dev,
        })

    res = run_bass_kernel_spmd(nc, in_maps, core_ids=list(range(NCORES)))
    _cache["last_res"] = res
    out = np.zeros((BT, D), dtype=np.float32)
    for c in range(NCORES):
        out += np.asarray(res.results[c]["o"]).astype(np.float32)
    return out.reshape(B, T, D)


# revision 27
# speedup vs baseline: 1.0266x; 1.0266x over previous
"""Multi-head attention (B=2, T=2048, D=1024, 16 heads) on 8 TRN2 NeuronCores.

Sharding: tensor-parallel over heads (2 heads/core). Each core computes
Q/K/V projections for its 2 heads (full sequence), causal attention in the
S^T = K @ Q^T form, and a partial output projection; the host sums the 8
partial outputs.

v2 redesign vs baseline (293us):
- all matmul operands bf16 (same 1 cycle/row as f32r but FWL weight loads
  and half the HBM traffic; rel-err budget 2e-2 >> bf16 error)
- causal masking via gpsimd affine_select post-exp zeroing (idle engine)
  instead of additive mask tensors on DVE; diagonal blocks are also
  extent-trimmed (S/exp/AV only computed on the valid column range)
- V transposed into [tk, d] layout by DMA XBAR transpose, not PE transpose
- softmax denominators: reciprocal_approx_fast on [1,512] rows + gpsimd
  partition_broadcast (no DRAM bounce)
- two clean phases that each fit the 8 PSUM banks exactly: (1) all QKV
  projections, (2) attention with o-proj steps interleaved via deferral
"""

import sys

sys.path.insert(0, "/opt/trn_rl_repo")

import numpy as np

B, T, D = 2, 2048, 1024
NCORES = 8
DV = 128  # head dims per core (2 heads x 64)
DH = 64
BT = B * T
CH = 512  # tq chunk width
NCH = BT // CH  # 8 global chunks
NCH_B = T // CH  # 4 chunks per batch
TK = 128  # tk tile
NTK = T // TK  # 16 tiles per batch
ND = D // 128  # 8 contraction tiles
DVA = DH + 1  # V columns incl ones column (for softmax denominator)

_cache = {}


def _build(debug=False):
    import concourse.bacc as bacc
    import concourse.mybir as mybir
    import concourse.tile as tile

    F32 = mybir.dt.float32
    BF16 = mybir.dt.bfloat16
    EXP = mybir.ActivationFunctionType.Exp
    MULT = mybir.AluOpType.mult
    from concourse.masks import make_identity

    nc = bacc.Bacc("TRN2", target_bir_lowering=False, debug=False,
                   num_devices=NCORES)

    xt_d = nc.dram_tensor("xt", [D, BT], BF16, kind="ExternalInput").ap()
    wq_d = nc.dram_tensor("wq", [128, D], BF16, kind="ExternalInput").ap()
    wk_d = nc.dram_tensor("wk", [128, D], BF16, kind="ExternalInput").ap()
    wv_d = nc.dram_tensor("wv", [128, D], BF16, kind="ExternalInput").ap()
    wo_d = nc.dram_tensor("wo", [128, D], BF16, kind="ExternalInput").ap()
    o_d = nc.dram_tensor("o", [BT, D], BF16, kind="ExternalOutput").ap()

    NU = B * NTK * 2  # 64 per-head V blocks
    if debug:
        qT_dbg = nc.dram_tensor("qT_dbg", [128, BT], BF16,
                                kind="ExternalOutput").ap()
        kT_dbg = nc.dram_tensor("kT_dbg", [128, BT], BF16,
                                kind="ExternalOutput").ap()
        vsb_dbg = nc.dram_tensor("vsb_dbg", [128, NU * DVA], BF16,
                                 kind="ExternalOutput").ap()
        outT_dbg = nc.dram_tensor("outT_dbg", [128, BT], BF16,
                                  kind="ExternalOutput").ap()
        srow_dbg = nc.dram_tensor("srow_dbg", [2, NCH, CH], F32,
                                  kind="ExternalOutput").ap()
        rbc_dbg = nc.dram_tensor("rbc_dbg", [128, NCH, CH], F32,
                                 kind="ExternalOutput").ap()
        p_dbg = nc.dram_tensor("p_dbg", [128, 4, 2, CH], BF16,
                               kind="ExternalOutput").ap()

    with tile.TileContext(nc) as tc:
        with tc.tile_pool(name="consts", bufs=1) as consts, \
             tc.tile_pool(name="perm", bufs=1) as perm, \
             tc.tile_pool(name="xt_pool", bufs=4) as xtp, \
             tc.tile_pool(name="vtf_pool", bufs=2) as vtfp, \
             tc.tile_pool(name="p_pool", bufs=4) as ppool, \
             tc.tile_pool(name="avc_pool", bufs=2) as avcp, \
             tc.tile_pool(name="rec_pool", bufs=2) as recp, \
             tc.tile_pool(name="outT_pool", bufs=2) as outTp, \
             tc.tile_pool(name="osb_pool", bufs=3) as obp, \
             tc.tile_pool(name="dram_pool", bufs=2, space="DRAM") as drp:
            wq_sb = consts.tile([128, D], BF16, name="wq_sb")
            wk_sb = consts.tile([128, D], BF16, name="wk_sb")
            wv_sb = consts.tile([128, D], BF16, name="wv_sb")
            wo_sb = consts.tile([128, D], BF16, name="wo_sb")
            ident = consts.tile([128, 128], BF16, name="ident")
            make_identity(nc, ident[:])

            qT = perm.tile([128, BT], BF16, name="qT")
            kT = perm.tile([128, BT], BF16, name="kT")
            # V blocks, t-major with ones column: per (b, tile, head) a
            # [128(tk), 65] block at free offset u*65, u = (b*NTK+i)*2+h
            vsb = perm.tile([128, NU * DVA], BF16, name="vsb")
            nc.gpsimd.memset(
                vsb[:].rearrange("p (u c) -> p u c", c=DVA)[:, :, DH:DVA], 1.0)


            # ---------------- Phase 1: all QKV projections ----------------
            for jp in range(NCH // 2):
                j0 = 2 * jp
                with tc.tile_pool(name=f"proj_ps{jp}", bufs=1,
                                  space="PSUM") as pps, \
                     tc.tile_pool(name=f"vt_ps{jp}", bufs=2,
                                  space="PSUM") as vtps:
                    acc = {}
                    for nm in ("q", "k", "v"):
                        for half in (0, 1):
                            acc[(nm, half)] = pps.tile(
                                [128, CH], F32, tag=f"{nm}{half}",
                                name=f"{nm}ps{j0 + half}")
                    for d in range(ND):
                        xt = xtp.tile([128, 2 * CH], BF16, tag="xt",
                                      name=f"xt{jp}_{d}")
                        nc.sync.dma_start(
                            xt[:], xt_d[d * 128:(d + 1) * 128,
                                        j0 * CH:(j0 + 2) * CH])
                        st, sp = d == 0, d == ND - 1
                        ws = slice(d * 128, (d + 1) * 128)
                        if jp == 0:
                            nc.sync.dma_start(wv_sb[:, ws], wv_d[:, ws])
                            nc.sync.dma_start(wq_sb[:, ws], wq_d[:, ws])
                            nc.sync.dma_start(wk_sb[:, ws], wk_d[:, ws])
                        for nm, w_sb in (("v", wv_sb), ("q", wq_sb),
                                         ("k", wk_sb)):
                            for half in (0, 1):
                                nc.tensor.matmul(
                                    acc[(nm, half)][:], w_sb[:, ws],
                                    xt[:, half * CH:(half + 1) * CH],
                                    start=st, stop=sp)
                    for half in (0, 1):
                        j = j0 + half
                        cs = slice(j * CH, (j + 1) * CH)
                        vtf = vtfp.tile([128, CH], BF16, tag="vtf",
                                        name=f"vtf{j}")
                        for q4 in range(4):
                            q4s = slice(q4 * 128, (q4 + 1) * 128)
                            if half == 0:
                                nc.vector.tensor_copy(vtf[:, q4s],
                                                      acc[("v", half)][:, q4s])
                            else:
                                nc.scalar.copy(vtf[:, q4s],
                                               acc[("v", half)][:, q4s])
                        nc.vector.tensor_copy(qT[:, cs], acc[("q", half)][:])
                        nc.scalar.copy(kT[:, cs], acc[("k", half)][:])
                        for tt in range(4):
                            tglob = 4 * j + tt
                            bb, ii = tglob // NTK, tglob % NTK
                            u0 = (bb * NTK + ii) * 2
                            vt_ps = vtps.tile([128, 128], BF16, tag="vt",
                                              name=f"vt{tglob}")
                            nc.tensor.transpose(
                                vt_ps[:], vtf[:, tt * 128:(tt + 1) * 128],
                                ident[:])
                            dst = vsb[:, u0 * DVA:(u0 + 2) * DVA].rearrange(
                                "p (h c) -> p h c", c=DVA)[:, :, 0:DH]
                            nc.vector.tensor_copy(
                                dst, vt_ps[:].rearrange("p (h c) -> p h c",
                                                        c=DH))
                if jp == 0:
                    nc.sync.dma_start(wo_sb[:], wo_d[:])

            # ---------------- Phase 2: attention + o-proj ----------------
            deferred = []

            def emit_oproj(b, jj, outT):
                for tt in range(4):
                    osb = obp.tile([128, D], BF16, tag="osb",
                                   name=f"osb{b}_{jj}_{tt}")
                    for half in (0, 1):
                        def step(opsp, tt=tt, half=half, b=b, jj=jj,
                                 outT=outT, osb=osb):
                            op = opsp.tile([128, CH], F32, tag="op",
                                           name=f"op{b}_{jj}_{tt}_{half}")
                            ts = slice(tt * 128, (tt + 1) * 128)
                            hs = slice(half * CH, (half + 1) * CH)
                            nc.tensor.matmul(op[:], outT[:, ts],
                                             wo_sb[:, hs],
                                             start=True, stop=True)
                            nc.vector.tensor_copy(osb[:, hs], op[:])
                            if half == 1:
                                r0 = b * T + jj * CH + tt * 128
                                nc.sync.dma_start(o_d[r0:r0 + 128, :],
                                                  osb[:])
                        deferred.append(step)

            def attention_chunk(b, jj, spsp, avp, opsp):
                # diagonal blocks first: their post-exp affine_select
                # latency is absorbed by the AV-chain slack instead of
                # extending the chunk tail. r=0 (full width) must be first
                # so the av accumulation group starts with a full write.
                kept = list(range(4 * jj, 4 * jj + 4)) + list(range(4 * jj))
                av0 = avp.tile([DVA, CH], F32, tag="av0", name=f"av0_{b}_{jj}",
                               bufs=2)
                av1 = avp.tile([DVA, CH], F32, tag="av1", name=f"av1_{b}_{jj}",
                               bufs=1)
                tq0 = (b * NCH_B + jj) * CH
                pend = None

                def emit_av(i, p, n0):
                    st = i == kept[0]
                    sp = i == kept[-1]
                    assert not (st and n0 > 0)
                    u0 = (b * NTK + i) * 2
                    nc.tensor.matmul(
                        av0[:, n0:CH], vsb[:, u0 * DVA:u0 * DVA + DVA],
                        p[:, 0, n0:CH], start=st, stop=sp)
                    nc.tensor.matmul(
                        av1[:, n0:CH], vsb[:, (u0 + 1) * DVA:(u0 + 2) * DVA],
                        p[:, 1, n0:CH], start=st, stop=sp)

                for i in kept:
                    r = i - 4 * jj  # diagonal sub-block index (>=0: diagonal)
                    n0 = 128 * r if r > 0 else 0  # first valid tq column
                    ks = slice((b * NTK + i) * TK, (b * NTK + i + 1) * TK)
                    sps = spsp.tile([128, 2, CH], F32, tag="sps",
                                    name=f"sps{b}_{jj}_{i}")
                    nc.tensor.matmul(sps[:, 0, n0:CH], kT[0:64, ks],
                                     qT[0:64, tq0 + n0:tq0 + CH],
                                     start=True, stop=True)
                    nc.tensor.matmul(sps[:, 1, n0:CH], kT[64:128, ks],
                                     qT[64:128, tq0 + n0:tq0 + CH],
                                     start=True, stop=True)
                    p = ppool.tile([128, 2, CH], BF16, tag="p",
                                   name=f"p{b}_{jj}_{i}")
                    nc.scalar.activation(p[:, :, n0:CH], sps[:, :, n0:CH],
                                         EXP)
                    if r >= 0:
                        # zero the strict upper triangle of the diagonal
                        # [128,128] sub-block: keep iff tk(partition) <= tq
                        nc.gpsimd.affine_select(
                            out=p[:, :, n0:n0 + 128],
                            in_=p[:, :, n0:n0 + 128],
                            compare_op=mybir.AluOpType.is_ge,
                            fill=0.0,
                            base=0,
                            pattern=[[0, 2], [1, 128]],
                            channel_multiplier=-1,
                        )
                    if debug and b == 0 and jj == 0:
                        nc.sync.dma_start(p_dbg[:, i, :, :], p[:])
                    if pend is not None:
                        emit_av(*pend)
                    if deferred:
                        deferred.pop(0)(opsp)
                        if len(deferred) > 6:
                            deferred.pop(0)(opsp)
                    pend = (i, p, n0)
                emit_av(*pend)

                # evacuate av banks (row DH holds the softmax denominators);
                # partition-crossing copies are legal, compute ops must stay
                # partition-aligned
                avc = avcp.tile([128, CH], F32, tag="avc",
                                name=f"avc_{b}_{jj}")
                srowA = avcp.tile([1, CH], F32, tag="srowA",
                                  name=f"srowA_{b}_{jj}")
                srowB = avcp.tile([1, CH], F32, tag="srowB",
                                  name=f"srowB_{b}_{jj}")
                # av1 is single-buffered: evacuate it first, on the scalar
                # engine, so the next chunk's AV can start promptly
                nc.vector.tensor_copy(avc[64:128, :], av1[0:DH, :])
                nc.vector.tensor_copy(srowB[:], av1[DH:DVA, :])
                nc.vector.tensor_copy(avc[0:64, :], av0[0:DH, :])
                nc.vector.tensor_copy(srowA[:], av0[DH:DVA, :])
                recA = recp.tile([1, CH], F32, tag="recA", name=f"recA{b}_{jj}")
                recB = recp.tile([1, CH], F32, tag="recB", name=f"recB{b}_{jj}")
                nc.vector.reciprocal_approx_fast(recA[:], srowA[:])
                nc.vector.reciprocal_approx_fast(recB[:], srowB[:])
                dr = drp.tile([2, CH], F32, tag="dr", name=f"dr_{b}_{jj}")
                nc.sync.dma_start(dr[0:1, :], recA[:])
                nc.sync.dma_start(dr[1:2, :], recB[:])
                rbc = recp.tile([128, CH], F32, tag="rbc", name=f"rbc{b}_{jj}")
                nc.sync.dma_start(rbc[0:64, :],
                                  dr[0:1, :].broadcast_to([64, CH]))
                nc.sync.dma_start(rbc[64:128, :],
                                  dr[1:2, :].broadcast_to([64, CH]))
                outT = outTp.tile([128, CH], BF16, tag="outT",
                                  name=f"outT{b}_{jj}")
                nc.gpsimd.tensor_tensor(out=outT[0:64, :], in0=avc[0:64, :],
                                         in1=rbc[0:64, :], op=MULT)
                nc.gpsimd.tensor_tensor(out=outT[64:128, :],
                                        in0=avc[64:128, :],
                                        in1=rbc[64:128, :], op=MULT)
                if debug:
                    cidx = b * NCH_B + jj
                    cs_ = slice(cidx * CH, (cidx + 1) * CH)
                    nc.sync.dma_start(outT_dbg[:, cs_], outT[:])
                    nc.sync.dma_start(srow_dbg[0:1, cidx], srowA[:])
                    nc.sync.dma_start(srow_dbg[1:2, cidx], srowB[:])
                    nc.sync.dma_start(rbc_dbg[:, cidx], rbc[:])
                emit_oproj(b, jj, outT)

            with tc.tile_pool(name="s_ps", bufs=2, space="PSUM") as spsp, \
                 tc.tile_pool(name="av_ps", bufs=2, space="PSUM") as avp, \
                 tc.tile_pool(name="o_ps", bufs=1, space="PSUM") as opsp:
                for b in range(B):
                    for jj in range(NCH_B):
                        attention_chunk(b, jj, spsp, avp, opsp)
                while deferred:
                    deferred.pop(0)(opsp)
                if debug:
                    nc.sync.dma_start(qT_dbg[:], qT[:])
                    nc.sync.dma_start(kT_dbg[:], kT[:])
                    nc.sync.dma_start(vsb_dbg[:], vsb[:])

    nc.compile()
    return nc


def kernel(x, Wq, Wk, Wv, Wo, attn_mask):
    import concourse.bass_utils as _bu
    import ml_dtypes
    run_bass_kernel_spmd = _bu.run_bass_kernel_spmd
    BF = ml_dtypes.bfloat16

    x = np.asarray(x, dtype=np.float32)
    Wq = np.asarray(Wq, dtype=np.float32)
    Wk = np.asarray(Wk, dtype=np.float32)
    Wv = np.asarray(Wv, dtype=np.float32)
    Wo = np.asarray(Wo, dtype=np.float32)

    xT = np.ascontiguousarray(x.reshape(BT, D).T).astype(BF)

    import os
    dbg = bool(os.environ.get("MHA_DEBUG"))
    if ("nc", dbg) not in _cache:
        _cache[("nc", dbg)] = _build(debug=dbg)
    nc = _cache[("nc", dbg)]

    in_maps = []
    for c in range(NCORES):
        rows = slice(c * DV, (c + 1) * DV)

        def wlayout(W, scale=1.0):
            Wc = W[rows, :]  # [128, D]
            return np.ascontiguousarray(
                (Wc.T.reshape(ND, 128, 128).transpose(1, 0, 2)
                 .reshape(128, D) * scale)).astype(BF)

        wo_dev = np.ascontiguousarray(Wo[:, rows].T).astype(BF)
        in_maps.append({
            "xt": xT,
            "wq": wlayout(Wq, 0.125),
            "wk": wlayout(Wk),
            "wv": wlayout(Wv),
            "wo": wo_dev,
        })

    res = run_bass_kernel_spmd(nc, in_maps, core_ids=list(range(NCORES)))
    _cache["last_res"] = res
    out = np.zeros((BT, D), dtype=np.float32)
    for c in range(NCORES):
        out += np.asarray(res.results[c]["o"]).astype(np.float32)
    return out.reshape(B, T, D)


# revision 28
# speedup vs baseline: 1.1179x; 1.0889x over previous
"""Multi-head attention (B=2, T=2048, D=1024, 16 heads) on 8 TRN2 NeuronCores.

Sharding: tensor-parallel over heads (2 heads/core). Each core computes
Q/K/V projections for its 2 heads (full sequence), causal attention in the
S^T = K @ Q^T form, and a partial output projection; the host sums the 8
partial outputs.

v7: fully interleaved schedule. Projection work for chunk-pair jp+1 is
split into units (one PSUM accumulation group of 8 matmuls + evacuation,
or one V-transpose) that are popped between attention blocks of chunks
2jp/2jp+1, so the tensor engine never idles during the ACT-bound exp
chain and the HAM clock gate stays at full speed. PSUM: one shared
2-slot ring for {proj accumulators, o-proj outputs, V transposes} (the
consumer of each slot is a fast evacuation copy), 4 banks for the S
double-buffer, 2 for the AV accumulators. bf16 everywhere; causal
masking via gpsimd affine_select on the diagonal blocks (processed
first, so the select latency hides in AV-chain slack); softmax
denominators via a ones-column in V + reciprocal_approx_fast.
"""

import sys

sys.path.insert(0, "/opt/trn_rl_repo")

import numpy as np

B, T, D = 2, 2048, 1024
NCORES = 8
DV = 128  # head dims per core (2 heads x 64)
DH = 64
BT = B * T
CH = 512  # tq chunk width
NCH = BT // CH  # 8 global chunks
NCH_B = T // CH  # 4 chunks per batch
TK = 128  # tk tile
NTK = T // TK  # 16 tiles per batch
ND = D // 128  # 8 contraction tiles
DVA = DH + 1  # V columns incl ones column (for softmax denominator)

_cache = {}


def _build(debug=False):
    import concourse.bacc as bacc
    import concourse.mybir as mybir
    import concourse.tile as tile

    F32 = mybir.dt.float32
    BF16 = mybir.dt.bfloat16
    EXP = mybir.ActivationFunctionType.Exp
    MULT = mybir.AluOpType.mult
    from concourse.masks import make_identity

    nc = bacc.Bacc("TRN2", target_bir_lowering=False, debug=False,
                   num_devices=NCORES)

    xt_d = nc.dram_tensor("xt", [D, BT], BF16, kind="ExternalInput").ap()
    wq_d = nc.dram_tensor("wq", [128, D], BF16, kind="ExternalInput").ap()
    wk_d = nc.dram_tensor("wk", [128, D], BF16, kind="ExternalInput").ap()
    wv_d = nc.dram_tensor("wv", [128, D], BF16, kind="ExternalInput").ap()
    wo_d = nc.dram_tensor("wo", [128, D], BF16, kind="ExternalInput").ap()
    o_d = nc.dram_tensor("o", [BT, D], BF16, kind="ExternalOutput").ap()

    NU = B * NTK * 2  # 64 per-head V blocks

    with tile.TileContext(nc) as tc:
        with tc.tile_pool(name="consts", bufs=1) as consts, \
             tc.tile_pool(name="perm", bufs=1) as perm, \
             tc.tile_pool(name="xt_pool", bufs=9) as xtp, \
             tc.tile_pool(name="vtf_pool", bufs=2) as vtfp, \
             tc.tile_pool(name="p_pool", bufs=4) as ppool, \
             tc.tile_pool(name="avc_pool", bufs=2) as avcp, \
             tc.tile_pool(name="rec_pool", bufs=2) as recp, \
             tc.tile_pool(name="outT_pool", bufs=2) as outTp, \
             tc.tile_pool(name="osb_pool", bufs=3) as obp, \
             tc.tile_pool(name="acc_ps", bufs=2, space="PSUM") as accp, \
             tc.tile_pool(name="s_ps", bufs=2, space="PSUM") as spsp, \
             tc.tile_pool(name="av_ps", bufs=1, space="PSUM") as avp, \
             tc.tile_pool(name="dram_pool", bufs=2, space="DRAM") as drp:
            wq_sb = consts.tile([128, D], BF16, name="wq_sb")
            wk_sb = consts.tile([128, D], BF16, name="wk_sb")
            wv_sb = consts.tile([128, D], BF16, name="wv_sb")
            wo_sb = consts.tile([128, D], BF16, name="wo_sb")
            ident = consts.tile([128, 128], BF16, name="ident")
            make_identity(nc, ident[:])

            qT = perm.tile([128, BT], BF16, name="qT")
            kT = perm.tile([128, BT], BF16, name="kT")
            # V blocks, t-major with ones column: per (b, tile, head) a
            # [128(tk), 65] block at free offset u*65, u = (b*NTK+i)*2+h
            vsb = perm.tile([128, NU * DVA], BF16, name="vsb")
            nc.gpsimd.memset(
                vsb[:].rearrange("p (u c) -> p u c", c=DVA)[:, :, DH:DVA], 1.0)

            # ---------------- projection work units ----------------
            units = []

            def emit_proj_pair(jp):
                j0 = 2 * jp
                xts = {}

                def u_dma(jp=jp, j0=j0, xts=xts):
                    for d in range(ND):
                        xt = xtp.tile([128, 2 * CH], BF16, tag="xt",
                                      name=f"xt{jp}_{d}")
                        nc.sync.dma_start(
                            xt[:], xt_d[d * 128:(d + 1) * 128,
                                        j0 * CH:(j0 + 2) * CH])
                        xts[d] = xt
                        if jp == 0:
                            ws = slice(d * 128, (d + 1) * 128)
                            nc.sync.dma_start(wv_sb[:, ws], wv_d[:, ws])
                            nc.sync.dma_start(wq_sb[:, ws], wq_d[:, ws])
                            nc.sync.dma_start(wk_sb[:, ws], wk_d[:, ws])
                units.append(u_dma)

                for half in (0, 1):
                    for nm in ("v", "q", "k"):
                        def u_group(nm=nm, half=half, jp=jp, j0=j0, xts=xts):
                            j = j0 + half
                            a = accp.tile([128, CH], F32, tag="acc",
                                          name=f"a_{nm}{j}")
                            w_sb = {"q": wq_sb, "k": wk_sb, "v": wv_sb}[nm]
                            for d in range(ND):
                                ws = slice(d * 128, (d + 1) * 128)
                                nc.tensor.matmul(
                                    a[:], w_sb[:, ws],
                                    xts[d][:, half * CH:(half + 1) * CH],
                                    start=d == 0, stop=d == ND - 1)
                            cs = slice(j * CH, (j + 1) * CH)
                            if nm == "q":
                                nc.vector.tensor_copy(qT[:, cs], a[:])
                            elif nm == "k":
                                nc.scalar.copy(kT[:, cs], a[:])
                            else:
                                vtf = vtfp.tile([128, CH], BF16, tag="vtf",
                                                name=f"vtf{j}")
                                nc.vector.tensor_copy(vtf[:], a[:])
                                xts[("vtf", half)] = vtf
                        units.append(u_group)
                    if half == 0 and jp == 0:
                        def u_wo():
                            nc.sync.dma_start(wo_sb[:], wo_d[:])
                        units.append(u_wo)

                for half in (0, 1):
                    for tt in range(4):
                        def u_vt(half=half, tt=tt, j0=j0, xts=xts):
                            j = j0 + half
                            tglob = 4 * j + tt
                            bb, ii = tglob // NTK, tglob % NTK
                            u0 = (bb * NTK + ii) * 2
                            vtf = xts[("vtf", half)]
                            vt_ps = accp.tile([128, 128], BF16, tag="acc",
                                              name=f"vt{tglob}")
                            nc.tensor.transpose(
                                vt_ps[:], vtf[:, tt * 128:(tt + 1) * 128],
                                ident[:])
                            dst = vsb[:, u0 * DVA:(u0 + 2) * DVA].rearrange(
                                "p (h c) -> p h c", c=DVA)[:, :, 0:DH]
                            nc.vector.tensor_copy(
                                dst, vt_ps[:].rearrange(
                                    "p (h c) -> p h c", c=DH))
                        units.append(u_vt)

            # ---------------- o-proj deferral ----------------
            deferred = []

            def emit_oproj(b, jj, outT):
                for tt in range(4):
                    osb = obp.tile([128, D], BF16, tag="osb",
                                   name=f"osb{b}_{jj}_{tt}")
                    for half in (0, 1):
                        def step(tt=tt, half=half, b=b, jj=jj,
                                 outT=outT, osb=osb):
                            op = accp.tile([128, CH], F32, tag="acc",
                                           name=f"op{b}_{jj}_{tt}_{half}")
                            ts = slice(tt * 128, (tt + 1) * 128)
                            hs = slice(half * CH, (half + 1) * CH)
                            nc.tensor.matmul(op[:], outT[:, ts],
                                             wo_sb[:, hs],
                                             start=True, stop=True)
                            nc.vector.tensor_copy(osb[:, hs], op[:])
                            if half == 1:
                                r0 = b * T + jj * CH + tt * 128
                                nc.sync.dma_start(o_d[r0:r0 + 128, :],
                                                  osb[:])
                        deferred.append(step)

            def pop_work():
                if units:
                    units.pop(0)()
                    if len(units) > 8:
                        units.pop(0)()
                elif deferred:
                    deferred.pop(0)()

            # ---------------- attention ----------------
            def attention_chunk(b, jj):
                # diagonal blocks first (r=0 full-width leads the av
                # accumulation; select latency absorbs into AV slack)
                kept = list(range(4 * jj, 4 * jj + 4)) + list(range(4 * jj))
                av0 = avp.tile([DVA, CH], F32, tag="av0",
                               name=f"av0_{b}_{jj}")
                av1 = avp.tile([DVA, CH], F32, tag="av1",
                               name=f"av1_{b}_{jj}")
                tq0 = (b * NCH_B + jj) * CH
                pend = None

                def emit_av(i, p, n0):
                    st = i == kept[0]
                    sp = i == kept[-1]
                    u0 = (b * NTK + i) * 2
                    nc.tensor.matmul(
                        av0[:, n0:CH], vsb[:, u0 * DVA:u0 * DVA + DVA],
                        p[:, 0, n0:CH], start=st, stop=sp)
                    nc.tensor.matmul(
                        av1[:, n0:CH], vsb[:, (u0 + 1) * DVA:(u0 + 2) * DVA],
                        p[:, 1, n0:CH], start=st, stop=sp)

                for i in kept:
                    r = i - 4 * jj  # diagonal sub-block index (>=0: diag)
                    n0 = 128 * r if r > 0 else 0  # first valid tq column
                    ks = slice((b * NTK + i) * TK, (b * NTK + i + 1) * TK)
                    sps = spsp.tile([128, 2, CH], F32, tag="sps",
                                    name=f"sps{b}_{jj}_{i}")
                    nc.tensor.matmul(sps[:, 0, n0:CH], kT[0:64, ks],
                                     qT[0:64, tq0 + n0:tq0 + CH],
                                     start=True, stop=True)
                    nc.tensor.matmul(sps[:, 1, n0:CH], kT[64:128, ks],
                                     qT[64:128, tq0 + n0:tq0 + CH],
                                     start=True, stop=True)
                    p = ppool.tile([128, 2, CH], BF16, tag="p",
                                   name=f"p{b}_{jj}_{i}")
                    nc.scalar.activation(p[:, :, n0:CH], sps[:, :, n0:CH],
                                         EXP)
                    if r >= 0:
                        # zero the strict upper triangle of the diagonal
                        # [128,128] sub-block: keep iff tk(partition) <= tq
                        nc.gpsimd.affine_select(
                            out=p[:, :, n0:n0 + 128],
                            in_=p[:, :, n0:n0 + 128],
                            compare_op=mybir.AluOpType.is_ge,
                            fill=0.0,
                            base=0,
                            pattern=[[0, 2], [1, 128]],
                            channel_multiplier=-1,
                        )
                    if pend is not None:
                        emit_av(*pend)
                    pop_work()
                    pend = (i, p, n0)
                emit_av(*pend)

                # evacuate av banks; row DH holds the softmax denominators
                avc = avcp.tile([128, CH], F32, tag="avc",
                                name=f"avc_{b}_{jj}")
                srow = avcp.tile([33, CH], F32, tag="srow",
                                 name=f"srow_{b}_{jj}")
                nc.vector.tensor_copy(avc[64:128, :], av1[0:DH, :])
                nc.vector.tensor_copy(srow[32:33, :], av1[DH:DVA, :])
                nc.vector.tensor_copy(avc[0:64, :], av0[0:DH, :])
                nc.vector.tensor_copy(srow[0:1, :], av0[DH:DVA, :])
                rec = recp.tile([33, CH], F32, tag="rec", name=f"rec{b}_{jj}")
                # one instr covers both sums rows (0 and 32); rows 1..31
                # are don't-care garbage
                nc.vector.reciprocal_approx_fast(rec[0:33, :], srow[0:33, :])
                dr = drp.tile([2, CH], F32, tag="dr", name=f"dr_{b}_{jj}")
                nc.sync.dma_start(dr[0:1, :], rec[0:1, :])
                nc.sync.dma_start(dr[1:2, :], rec[32:33, :])
                rbc = recp.tile([128, CH], F32, tag="rbc", name=f"rbc{b}_{jj}")
                nc.sync.dma_start(rbc[0:64, :],
                                  dr[0:1, :].broadcast_to([64, CH]))
                nc.sync.dma_start(rbc[64:128, :],
                                  dr[1:2, :].broadcast_to([64, CH]))
                outT = outTp.tile([128, CH], BF16, tag="outT",
                                  name=f"outT{b}_{jj}")
                nc.gpsimd.tensor_tensor(out=outT[0:64, :], in0=avc[0:64, :],
                                        in1=rbc[0:64, :], op=MULT)
                nc.gpsimd.tensor_tensor(out=outT[64:128, :],
                                        in0=avc[64:128, :],
                                        in1=rbc[64:128, :], op=MULT)
                emit_oproj(b, jj, outT)

            # ---------------- main schedule ----------------
            emit_proj_pair(0)
            while units:  # prologue: pair 0 alone
                units.pop(0)()
            for c in range(NCH):
                b, jj = c // NCH_B, c % NCH_B
                if c % 2 == 0 and c // 2 + 1 < NCH // 2:
                    emit_proj_pair(c // 2 + 1)
                attention_chunk(b, jj)
                if c % 2 == 1:
                    while units:  # chunk c+1 needs its pair complete
                        units.pop(0)()
            while deferred:
                deferred.pop(0)()

    nc.compile()
    return nc


def kernel(x, Wq, Wk, Wv, Wo, attn_mask):
    import concourse.bass_utils as _bu
    import ml_dtypes
    run_bass_kernel_spmd = _bu.run_bass_kernel_spmd
    BF = ml_dtypes.bfloat16

    x = np.asarray(x, dtype=np.float32)
    Wq = np.asarray(Wq, dtype=np.float32)
    Wk = np.asarray(Wk, dtype=np.float32)
    Wv = np.asarray(Wv, dtype=np.float32)
    Wo = np.asarray(Wo, dtype=np.float32)

    xT = np.ascontiguousarray(x.reshape(BT, D).T).astype(BF)

    if "nc" not in _cache:
        _cache["nc"] = _build()
    nc = _cache["nc"]

    in_maps = []
    for c in range(NCORES):
        rows = slice(c * DV, (c + 1) * DV)

        def wlayout(W, scale=1.0):
            Wc = W[rows, :]  # [128, D]
            return np.ascontiguousarray(
                (Wc.T.reshape(ND, 128, 128).transpose(1, 0, 2)
                 .reshape(128, D) * scale)).astype(BF)

        wo_dev = np.ascontiguousarray(Wo[:, rows].T).astype(BF)
        in_maps.append({
            "xt": xT,
            "wq": wlayout(Wq, 0.125),
            "wk": wlayout(Wk),
            "wv": wlayout(Wv),
            "wo": wo_dev,
        })

    res = run_bass_kernel_spmd(nc, in_maps, core_ids=list(range(NCORES)))
    _cache["last_res"] = res
    out = np.zeros((BT, D), dtype=np.float32)
    for c in range(NCORES):
        out += np.asarray(res.results[c]["o"]).astype(np.float32)
    return out.reshape(B, T, D)
